# revision 1
# baseline (speedup 1.0000x reference)
"""Trainium2 Bass kernel for nn_DiarizationLoss (PIT diarization loss).

Strategy (8 NeuronCores, T-sharded data-parallel):
  - Shard T=65536 into 8 slices of TLOC=8192; every core processes all B=32
    samples for its T-slice. Perfectly balanced, one SPMD program.
  - Rewrite the masked pairwise BCE cost + VAD BCE as pure dot products
    over t, computed as ONE packed TensorEngine contraction per core:
      rows (lhsT, bf16):  [lp_0..3, lq_0..3, lpv, lqv]   (Ln via ACT engine)
      cols (rhs,  bf16):  [mt_0..3, mask, vmask]         (DVE compare/mult)
    where lp=ln(p+eps), lq=ln((1+eps)-p), mt=labels*mask, vmask=vad*mask,
    mask[t] = (t < len_b) built on-device from an iota table and per-core
    thresholds.  8 samples are packed per matmul (lhsT [128,80] x rhs
    [128,48]) and 64 chunks PSUM-accumulate, so the PE does all heavy
    reduction work.  All DMA / ACT / DVE work is batched per 8-sample group
    (few large instructions - HWDGE issue overhead and per-op engine
    overheads dominate otherwise).
  - Host combines the tiny per-core partial-sum blocks: PIT permutation min
    over the 4x4 cost matrices, means, and the VAD quotient.

Layout per sample on a core: t_loc = 64*p + q  (p partition, q in [0,64)).
LHS tile c-major per sample: column c occupies [s*640 + c*64, +64) so the
packed matmul AP is a single free dim [[64, 80]] offset q (HW requirement:
the stationary matmul operand AP must have exactly one free dimension).
"""

import warnings

warnings.filterwarnings("ignore")

from contextlib import ExitStack
from itertools import permutations

import ml_dtypes
import numpy as np

import concourse.bass as bass
import concourse.mybir as mybir
import concourse.tile as tile
from concourse import bacc
from concourse.bass_utils import run_bass_kernel_spmd

F32 = mybir.dt.float32
BF16 = mybir.dt.bfloat16
U8 = mybir.dt.uint8
Ln = mybir.ActivationFunctionType.Ln
Alu = mybir.AluOpType

# problem constants (hardcoded per contract)
B, T, S = 32, 65536, 4
EPS = 1e-7
PIT_W, VAD_W = 1.0, 0.5
NCORES = 8
TLOC = T // NCORES          # 8192 timesteps per core
P = 128                     # partitions
Q = TLOC // P               # 64 free chunks per sample
GROUP = 8                   # samples packed per matmul
NG = B // GROUP             # 4 matmul groups
PERMS = np.array(list(permutations(range(S))), dtype=np.int64)  # [24, 4]

_CACHE = {}


def _build_nc(reps=1, loop_n=1):
    nc = bacc.Bacc("TRN2", target_bir_lowering=False, debug=False)

    # host pre-laid-out: ps fp32 [P,B*(q c)]; lb bf16 [P,B*(c q)];
    # pv fp32 [P,B*Q]; vd bf16 [P,B*Q]
    ps_d = nc.dram_tensor("ps", [P, B * Q * S], F32, kind="ExternalInput")
    lb_d = nc.dram_tensor("lb", [P, B * Q * S], U8, kind="ExternalInput")
    pv_d = nc.dram_tensor("pv", [P, B * Q], F32, kind="ExternalInput")
    vd_d = nc.dram_tensor("vd", [P, B * Q], U8, kind="ExternalInput")
    io1_d = nc.dram_tensor("io1", [P, Q], F32, kind="ExternalInput")
    thr_d = nc.dram_tensor("thr", [P, B + 2], F32, kind="ExternalInput")
    out_d = nc.dram_tensor("out", [NG, GROUP * 10, GROUP * 6], F32,
                           kind="ExternalOutput")

    with tile.TileContext(nc) as tc, ExitStack() as ctx:
        const_pool = ctx.enter_context(tc.tile_pool(name="const", bufs=1))
        stage_pool = ctx.enter_context(tc.tile_pool(name="stage", bufs=4))
        vstage_pool = ctx.enter_context(tc.tile_pool(name="vstage", bufs=1))
        lhs_pool = ctx.enter_context(tc.tile_pool(name="lhs", bufs=1))
        rhs_pool = ctx.enter_context(tc.tile_pool(name="rhs", bufs=1))
        psum_pool = ctx.enter_context(
            tc.tile_pool(name="psum", bufs=1, space="PSUM"))
        out_pool = ctx.enter_context(tc.tile_pool(name="outp", bufs=1))

        io1_t = const_pool.tile([P, Q], F32, tag="io1")
        thr_t = const_pool.tile([P, B + 2], F32, tag="thr")
        nc.sync.dma_start(io1_t[:], io1_d[:])
        nc.sync.dma_start(thr_t[:], thr_d[:])
        eps_ap = thr_t[:, B:B + 1]
        onep_ap = thr_t[:, B + 1:B + 2]

        lhs_ts, rhs_ts = [], []
        for g in range(NG):
            lhs_t = lhs_pool.tile([P, GROUP * Q * 10], BF16, tag=f"lhs{g}")
            rhs_t = rhs_pool.tile([P, GROUP * Q * 6], BF16, tag=f"rhs{g}")
            lhs_ts.append(lhs_t)
            rhs_ts.append(rhs_t)

        def build_pass():
            # all-sample VAD staging + masks
            pv_t = vstage_pool.tile([P, B * Q], F32, tag="pv")
            vd_t = vstage_pool.tile([P, B * Q], U8, tag="vd")
            msk_t = vstage_pool.tile([P, B * Q], BF16, tag="msk")
            nc.sync.dma_start(pv_t[:], pv_d[:])
            nc.gpsimd.dma_start(vd_t[:], vd_d[:])

            # prefetch every group's speaker data (ps on HWDGE, lb on SWDGE)
            ps_ts, lb_ts = [], []
            for g in range(NG):
                s0 = g * GROUP
                ps_t = stage_pool.tile([P, GROUP * Q * S], F32, tag="ps")
                nc.sync.dma_start(
                    ps_t[:], ps_d[:, s0 * Q * S:(s0 + GROUP) * Q * S])
                lb_t = stage_pool.tile([P, GROUP * Q * S], U8, tag="lb")
                nc.gpsimd.dma_start(
                    lb_t[:], lb_d[:, s0 * Q * S:(s0 + GROUP) * Q * S])
                ps_ts.append(ps_t)
                lb_ts.append(lb_t)

            # mask32[p, (b q)] = io1[p, q] < thr[p, b]
            nc.vector.tensor_tensor(
                msk_t[:].rearrange("p (b q) -> p b q", b=B, q=Q),
                io1_t[:].unsqueeze(1).broadcast_to([P, B, Q]),
                thr_t[:, :B].unsqueeze(2).broadcast_to([P, B, Q]),
                op=Alu.is_lt)
            msk_r = msk_t[:].rearrange("p (b q) -> p b q", b=B, q=Q)

            ot = out_pool.tile([GROUP * 10, NG * GROUP * 6], F32, tag="ot")
            for g in range(NG):
                s0 = g * GROUP
                lhs_r = lhs_ts[g][:].rearrange("p (s c q) -> p s c q",
                                               s=GROUP, c=10, q=Q)
                rhs_r = rhs_ts[g][:].rearrange("p (s c q) -> p s c q",
                                               s=GROUP, c=6, q=Q)

                ps_v = ps_ts[g][:].rearrange("p (s q c) -> p s c q",
                                             s=GROUP, q=Q, c=S)
                nc.scalar.activation(lhs_r[:, :, 0:4, :], ps_v, Ln,
                                     bias=eps_ap, scale=1.0)
                nc.scalar.activation(lhs_r[:, :, 4:8, :], ps_v, Ln,
                                     bias=onep_ap, scale=-1.0)
                nc.scalar.activation(
                    lhs_r[:, :, 8, :],
                    pv_t[:].rearrange("p (b q) -> p b q",
                                      b=B, q=Q)[:, s0:s0 + GROUP, :],
                    Ln, bias=eps_ap, scale=1.0)
                nc.scalar.activation(
                    lhs_r[:, :, 9, :],
                    pv_t[:].rearrange("p (b q) -> p b q",
                                      b=B, q=Q)[:, s0:s0 + GROUP, :],
                    Ln, bias=onep_ap, scale=-1.0)

                lb_v = lb_ts[g][:].rearrange("p (s c q) -> p s c q",
                                             s=GROUP, c=S, q=Q)
                # mt = labels * mask (mask broadcast over c)
                nc.vector.tensor_tensor(
                    rhs_r[:, :, 0:4, :], lb_v,
                    msk_r[:, s0:s0 + GROUP, :].unsqueeze(2)
                         .broadcast_to([P, GROUP, S, Q]),
                    op=Alu.mult)
                # mask -> bf16 rhs column
                nc.vector.tensor_copy(rhs_r[:, :, 4, :],
                                      msk_r[:, s0:s0 + GROUP, :])
                # vmask = vad * mask
                nc.vector.tensor_tensor(
                    rhs_r[:, :, 5, :],
                    vd_t[:].rearrange("p (b q) -> p b q",
                                      b=B, q=Q)[:, s0:s0 + GROUP, :],
                    msk_r[:, s0:s0 + GROUP, :],
                    op=Alu.mult)

                # matmul chain for this group
                lhs_f = lhs_ts[g][:]
                rhs_f = rhs_ts[g][:]
                acc = psum_pool.tile([GROUP * 10, GROUP * 6], F32,
                                     tag=f"acc{g}")
                for q in range(Q):
                    lhsT = bass.AP(lhs_f.tensor, lhs_f.offset + q,
                                   [list(lhs_f.ap[0]), [Q, GROUP * 10]])
                    rhs = bass.AP(rhs_f.tensor, rhs_f.offset + q,
                                  [list(rhs_f.ap[0]), [Q, GROUP * 6]])
                    nc.tensor.matmul(acc[:], lhsT, rhs,
                                     start=(q == 0), stop=(q == Q - 1))
                nc.vector.tensor_copy(
                    ot[:, g * GROUP * 6:(g + 1) * GROUP * 6], acc[:])

            nc.sync.dma_start(
                out_d[:].rearrange("g m n -> m g n"), ot[:].rearrange(
                    "m (g n) -> m g n", g=NG, n=GROUP * 6))

        # reps/loop_n > 1 only for timing-by-differencing in test.py
        if loop_n > 1:
            with tc.For_i(0, loop_n, 1):
                for _ in range(reps):
                    build_pass()
        else:
            for _ in range(reps):
                build_pass()

    nc.compile()
    return nc


def _get_nc(reps=1, loop_n=1):
    key = ("nc", reps, loop_n)
    if key not in _CACHE:
        _CACHE[key] = _build_nc(reps, loop_n)
    return _CACHE[key]


def _make_in_maps(pred_speakers, pred_vad, labels, vad, lengths):
    io1 = (np.arange(P)[:, None] * Q
           + np.arange(Q)[None, :]).astype(np.float32)
    lens = np.asarray(lengths, dtype=np.float64)
    in_maps = []
    for c in range(NCORES):
        t0 = c * TLOC
        thr = np.zeros((P, B + 2), np.float32)
        thr[:, :B] = (lens - t0).astype(np.float32)[None, :]
        thr[:, B] = EPS
        thr[:, B + 1] = 1.0 + EPS
        bf16 = ml_dtypes.bfloat16

        def lay3(x):  # [B, TLOC, S] -> [P, B*(q c)] fp32
            return np.ascontiguousarray(
                np.asarray(x, np.float32)[:, t0:t0 + TLOC, :]
                .reshape(B, P, Q * S).transpose(1, 0, 2)).reshape(P, B * Q * S)

        def lay3c(x):  # [B, TLOC, S] -> [P, B*(c q)] u8
            return np.ascontiguousarray(
                np.asarray(x)[:, t0:t0 + TLOC, :].astype(np.uint8)
                .reshape(B, P, Q, S).transpose(1, 0, 3, 2)).reshape(
                    P, B * Q * S)

        def lay2(x, dt):  # [B, TLOC] -> [P, B*Q]
            return np.ascontiguousarray(
                np.asarray(x).astype(dt)[:, t0:t0 + TLOC]
                .reshape(B, P, Q).transpose(1, 0, 2)).reshape(P, B * Q)

        in_maps.append({
            "ps": lay3(pred_speakers),
            "lb": lay3c(labels),
            "pv": lay2(pred_vad, np.float32),
            "vd": lay2(vad, np.uint8),
            "io1": io1,
            "thr": thr,
        })
    return in_maps


def _combine(outs, lengths):
    """Host reduction of per-core partial-sum blocks -> scalar loss."""
    tot = np.zeros((NG, GROUP * 10, GROUP * 6), np.float64)
    for o in outs:
        tot += o.astype(np.float64)

    lens = np.asarray(lengths, dtype=np.float64)
    speaker_sum = 0.0
    vad_num = 0.0
    for b in range(B):
        g, s = b // GROUP, b % GROUP
        blk = tot[g, 10 * s:10 * s + 10, 6 * s:6 * s + 6]
        P1 = blk[0:4, 0:4]          # sum lp_i * mt_j
        Q1 = blk[4:8, 0:4]          # sum lq_i * mt_j
        Q2 = blk[4:8, 4]            # sum lq_i * mask
        lpv_vm = blk[8, 5]          # sum lpv * vad * mask
        lqv_m = blk[9, 4]           # sum lqv * mask
        lqv_vm = blk[9, 5]          # sum lqv * vad * mask

        term1 = -(P1 - Q1)          # [4,4]
        term2 = -Q2                 # [4]
        msum = lens[b]
        L = (term1 + term2[:, None]) / msum
        perm_losses = L[np.arange(S)[None, :], PERMS].mean(axis=-1)  # [24]
        speaker_sum += perm_losses.min()

        vad_num += -(lpv_vm + lqv_m - lqv_vm)

    speaker_loss = speaker_sum / B
    vad_loss = vad_num / lens.sum()
    return np.float32(PIT_W * speaker_loss + VAD_W * vad_loss)


def kernel(pred_speakers, pred_vad, labels, vad, lengths):
    nc = _get_nc()
    in_maps = _make_in_maps(pred_speakers, pred_vad, labels, vad, lengths)
    res = run_bass_kernel_spmd(nc, in_maps, core_ids=list(range(NCORES)))
    outs = [res.results[c]["out"] for c in range(NCORES)]
    return _combine(outs, lengths)


if __name__ == "__main__":
    rng = np.random.default_rng(0)
    inputs = {
        "pred_speakers": rng.random((B, T, S), np.float32),
        "pred_vad": rng.random((B, T), np.float32),
        "labels": rng.integers(0, 2, (B, T, S)).astype(np.float32),
        "vad": rng.integers(0, 2, (B, T)).astype(np.float32),
        "lengths": np.maximum(rng.integers(0, T, B), T // 2).astype(np.int64),
    }
    print("loss:", kernel(**inputs))



# revision 17
# speedup vs baseline: 1.0770x; 1.0770x over previous
"""Trainium2 Bass kernel for nn_DiarizationLoss (PIT diarization loss).

Strategy (8 NeuronCores, T-sharded data-parallel):
  - Shard T=65536 into 8 slices of TLOC=8192; every core processes all B=32
    samples for its T-slice.  t_loc = 64*p + q  (p partition, q in [0,64)).
  - The masked pairwise BCE cost + VAD BCE reduce to per-sample dot products
    over t.  Per sample the device computes, via TensorEngine PSUM
    accumulation over 64 q-chunks of K=128 partitions:
      stationary (5 cols): [mt_0..3, mask]      mt = labels*mask
      moving     (9 cols): [lp_0..3, lnxv, lq_0..3]
    where lp=ln(p+eps), lq=ln((1+eps)-p), xv=|(1-vad)-pv| (== pv if vad else
    1-pv), lnxv=ln(xv+eps).  16 samples pack per matmul group (stationary
    [128,80] x moving [128,144] -> PSUM [80,144]); 2 groups cover B=32.
  - ScalarE computes all Ln columns (the bottleneck engine); the VAD column
    rides inside the lp activation op via a spare 5th pred column.  DVE does
    the label masking and the |offv-pv| select.  Labels+mask+offv ship as one
    u8 tensor and are cast to bf16 by the SWDGE DMA.
  - Host combines the tiny per-core partial-sum blocks: PIT permutation min
    over the 4x4 cost matrices, means, and the VAD quotient.
"""

import warnings

warnings.filterwarnings("ignore")

from contextlib import ExitStack
from itertools import permutations

import ml_dtypes
import numpy as np

import concourse.bass as bass
import concourse.mybir as mybir
import concourse.tile as tile
from concourse import bacc
from concourse.bass_utils import run_bass_kernel_spmd

F32 = mybir.dt.float32
BF16 = mybir.dt.bfloat16
U8 = mybir.dt.uint8
U16 = mybir.dt.uint16
SC = 1.0 / 65536.0          # u16 fixed-point scale, applied inside ACT
Ln = mybir.ActivationFunctionType.Ln
Alu = mybir.AluOpType

# problem constants (hardcoded per contract)
B, T, S = 32, 65536, 4
EPS = 1e-7
PIT_W, VAD_W = 1.0, 0.5
NCORES = 8
TLOC = T // NCORES          # 8192 timesteps per core
P = 128                     # partitions
Q = TLOC // P               # 64 free chunks per sample
G = 16                      # samples per matmul group
NG = B // G                 # 2 matmul groups
LC = 5                      # stationary cols per sample: mt0..3, mask
MC = 9                      # moving cols per sample: lp0..3, lnxv, lq0..3
PERMS = np.array(list(permutations(range(S))), dtype=np.int64)  # [24, 4]

_CACHE = {}


def _build_nc(reps=1, loop_n=1):
    nc = bacc.Bacc("TRN2", target_bir_lowering=False, debug=False)

    # host pre-laid-out (see _make_in_maps):
    #   ps  u16 [P, B*(5 Q)]  cols 0..3 = preds*65536 (c-major), col 4 scratch
    #   lbm u8  [P, B*(5 Q)]  per-sample [lab0..3, mask]
    #   pv  u16 [P, B*Q]  pred_vad*65536;  off u16 [P, B*Q]  (1-vad)*65535
    ps_d = nc.dram_tensor("ps", [P, B * 5 * Q], U16, kind="ExternalInput")
    lbm_d = nc.dram_tensor("lbm", [P, B * 5 * Q], U8, kind="ExternalInput")
    pv_d = nc.dram_tensor("pv", [P, B * Q], U16, kind="ExternalInput")
    off_d = nc.dram_tensor("off", [P, B * Q], U16, kind="ExternalInput")
    cst_d = nc.dram_tensor("cst", [P, 2], F32, kind="ExternalInput")
    out_d = nc.dram_tensor("out", [G * LC, NG * G * MC], F32,
                           kind="ExternalOutput")

    with tile.TileContext(nc) as tc, ExitStack() as ctx:
        const_pool = ctx.enter_context(tc.tile_pool(name="const", bufs=1))
        stage_pool = ctx.enter_context(tc.tile_pool(name="stage", bufs=2))
        work_pool = ctx.enter_context(tc.tile_pool(name="work", bufs=1))
        psum_pool = ctx.enter_context(
            tc.tile_pool(name="psum", bufs=2, space="PSUM"))
        out_pool = ctx.enter_context(tc.tile_pool(name="outp", bufs=2))

        lhs_ts = [work_pool.tile([P, G * MC * Q], BF16, tag=f"lhs{g}",
                                 name=f"lhs{g}") for g in range(NG)]
        rhs_ts = [work_pool.tile([P, G * LC * Q], BF16, tag=f"rhs{g}",
                                 name=f"rhs{g}") for g in range(NG)]
        cst_t = const_pool.tile([P, 2], F32, tag="cst")
        nc.sync.dma_start(cst_t[:], cst_d[:])
        eps_ap = cst_t[:, 0:1]       # EPS
        onep_ap = cst_t[:, 1:2]      # 1 + EPS

        def build_pass():
            ps_t = stage_pool.tile([P, B * 5 * Q], U16, tag="ps")
            lbm_t = stage_pool.tile([P, B * 5 * Q], BF16, tag="lbm")
            pv_t = stage_pool.tile([P, B * Q], U16, tag="pv")
            off_t = stage_pool.tile([P, B * Q], U16, tag="off")
            nc.sync.dma_start(ps_t[:], ps_d[:])
            nc.gpsimd.dma_start(lbm_t[:], lbm_d[:])   # u8 -> bf16 cast DMA
            nc.sync.dma_start(pv_t[:], pv_d[:])
            nc.sync.dma_start(off_t[:], off_d[:])

            ps_r = ps_t[:].rearrange("p (s c q) -> p s c q", s=B, c=5, q=Q)
            lbm_r = lbm_t[:].rearrange("p (s c q) -> p s c q", s=B, c=5, q=Q)

            ot = out_pool.tile([G * LC, NG * G * MC], F32, tag="ot")
            for g in range(NG):
                s0, s1 = g * G, (g + 1) * G
                lhs_r = lhs_ts[g][:].rearrange("p (s c q) -> p s c q",
                                               s=G, c=MC, q=Q)
                rhs_r = rhs_ts[g][:].rearrange("p (s c q) -> p s c q",
                                               s=G, c=LC, q=Q)

                # xv = pv XOR off  (off in {0x0000, 0xFFFF}; equals pv when
                # vad=1, 65535-pv when vad=0) into ps col 4 (Ln'd by lp op)
                nc.vector.tensor_tensor(
                    ps_r[:, s0:s1, 4, :],
                    pv_t[:, s0 * Q:s1 * Q].rearrange(
                        "p (s q) -> p s q", s=G, q=Q),
                    off_t[:, s0 * Q:s1 * Q].rearrange(
                        "p (s q) -> p s q", s=G, q=Q),
                    op=Alu.bitwise_xor)

                # mt = labels * mask ; mask copy
                nc.vector.tensor_tensor(
                    rhs_r[:, :, 0:4, :], lbm_r[:, s0:s1, 0:4, :],
                    lbm_r[:, s0:s1, 4:5, :].broadcast_to([P, G, 4, Q]),
                    op=Alu.mult)
                nc.vector.tensor_copy(rhs_r[:, :, 4, :],
                                      lbm_r[:, s0:s1, 4, :])

                # lp0..3 + lnxv, then lq0..3 (u16 in; scale 2^-16 inside ACT)
                nc.scalar.activation(lhs_r[:, :, 0:5, :], ps_r[:, s0:s1, :, :],
                                     Ln, bias=eps_ap, scale=SC)
                nc.scalar.activation(lhs_r[:, :, 5:9, :],
                                     ps_r[:, s0:s1, 0:4, :],
                                     Ln, bias=onep_ap, scale=-SC)

                # matmul chain: PSUM accumulate over 64 q-chunks
                lhs_f, rhs_f = lhs_ts[g][:], rhs_ts[g][:]
                acc = psum_pool.tile([G * LC, G * MC], F32, tag=f"acc{g}")
                for q in range(Q):
                    stat = bass.AP(rhs_f.tensor, rhs_f.offset + q,
                                   [list(rhs_f.ap[0]), [Q, G * LC]])
                    mov = bass.AP(lhs_f.tensor, lhs_f.offset + q,
                                  [list(lhs_f.ap[0]), [Q, G * MC]])
                    nc.tensor.matmul(acc[:], stat, mov,
                                     start=(q == 0), stop=(q == Q - 1))
                nc.vector.tensor_copy(
                    ot[:, g * G * MC:(g + 1) * G * MC], acc[:])

            nc.sync.dma_start(out_d[:], ot[:])

        # reps/loop_n > 1 only for timing-by-differencing in test.py
        if loop_n > 1:
            with tc.For_i(0, loop_n, 1):
                for _ in range(reps):
                    build_pass()
        else:
            for _ in range(reps):
                build_pass()

    nc.compile()
    return nc


def _get_nc(reps=1, loop_n=1):
    key = ("nc", reps, loop_n)
    if key not in _CACHE:
        _CACHE[key] = _build_nc(reps, loop_n)
    return _CACHE[key]


def _quant16(x):
    # fixed-point u16: round(x * 65536) clipped to [0, 65535]
    return np.minimum(np.rint(x * 65536.0), 65535.0).astype(np.uint16)


def _make_in_maps(pred_speakers, pred_vad, labels, vad, lengths):
    lens = np.asarray(lengths, dtype=np.int64)
    ps_all = np.asarray(pred_speakers, np.float32)
    pv_all = np.asarray(pred_vad, np.float32)
    lb_all = np.asarray(labels)
    vd_all = np.asarray(vad)

    in_maps = []
    for c in range(NCORES):
        t0 = c * TLOC
        # [B, TLOC] validity mask for this core's T-slice (exact int math)
        tidx = t0 + np.arange(TLOC, dtype=np.int64)
        mask = (tidx[None, :] < lens[:, None]).astype(np.uint8)  # [B, TLOC]

        # ps: [B, TLOC, S] -> [P, B, 5, Q] u16, col 4 zero (device scratch)
        psc = ps_all[:, t0:t0 + TLOC, :].reshape(B, P, Q, S)
        ps = np.zeros((P, B, 5, Q), dtype=np.uint16)
        ps[:, :, 0:4, :] = _quant16(psc).transpose(1, 0, 3, 2)

        # lbm: u8 [lab0..3, mask] per sample
        lbc = lb_all[:, t0:t0 + TLOC, :].reshape(B, P, Q, S).astype(np.uint8)
        lbm = np.empty((P, B, 5, Q), np.uint8)
        lbm[:, :, 0:4, :] = lbc.transpose(1, 0, 3, 2)
        lbm[:, :, 4, :] = mask.reshape(B, P, Q).transpose(1, 0, 2)

        pv = np.ascontiguousarray(
            _quant16(pv_all[:, t0:t0 + TLOC])
            .reshape(B, P, Q).transpose(1, 0, 2)).reshape(P, B * Q)
        off = np.ascontiguousarray(
            ((1 - vd_all[:, t0:t0 + TLOC].astype(np.int64)) * 65535)
            .astype(np.uint16)
            .reshape(B, P, Q).transpose(1, 0, 2)).reshape(P, B * Q)

        cst = np.zeros((P, 2), np.float32)
        cst[:, 0] = EPS
        cst[:, 1] = 1.0 + EPS

        in_maps.append({
            "ps": np.ascontiguousarray(ps.reshape(P, B * 5 * Q)),
            "lbm": np.ascontiguousarray(lbm.reshape(P, B * 5 * Q)),
            "pv": pv,
            "off": off,
            "cst": cst,
        })
    return in_maps


def _combine(outs, lengths):
    """Host reduction of per-core partial-sum blocks -> scalar loss."""
    tot = np.zeros((G * LC, NG * G * MC), np.float64)
    for o in outs:
        tot += o.astype(np.float64)

    lens = np.asarray(lengths, dtype=np.float64)
    speaker_sum = 0.0
    vad_num = 0.0
    for b in range(B):
        g, s = b // G, b % G
        blk = tot[LC * s:LC * s + LC,
                  g * G * MC + MC * s:g * G * MC + MC * s + MC]
        P1 = blk[0:4, 0:4].T        # [i, j] = sum lp_i * mt_j
        Q1 = blk[0:4, 5:9].T        # [i, j] = sum lq_i * mt_j
        Q2 = blk[4, 5:9]            # [i]    = sum lq_i * mask
        vad_num += -blk[4, 4]       # sum mask * ln(xv + eps)

        term1 = -(P1 - Q1)          # [4, 4]
        term2 = -Q2                 # [4]
        L = (term1 + term2[:, None]) / lens[b]
        perm_losses = L[np.arange(S)[None, :], PERMS].mean(axis=-1)  # [24]
        speaker_sum += perm_losses.min()

    speaker_loss = speaker_sum / B
    vad_loss = vad_num / lens.sum()
    return np.float32(PIT_W * speaker_loss + VAD_W * vad_loss)


def kernel(pred_speakers, pred_vad, labels, vad, lengths):
    nc = _get_nc()
    in_maps = _make_in_maps(pred_speakers, pred_vad, labels, vad, lengths)
    res = run_bass_kernel_spmd(nc, in_maps, core_ids=list(range(NCORES)))
    outs = [res.results[c]["out"] for c in range(NCORES)]
    return _combine(outs, lengths)


if __name__ == "__main__":
    rng = np.random.default_rng(0)
    inputs = {
        "pred_speakers": rng.random((B, T, S), np.float32),
        "pred_vad": rng.random((B, T), np.float32),
        "labels": rng.integers(0, 2, (B, T, S)).astype(np.float32),
        "vad": rng.integers(0, 2, (B, T)).astype(np.float32),
        "lengths": np.maximum(rng.integers(0, T, B), T // 2).astype(np.int64),
    }
    print("loss:", kernel(**inputs))


# revision 25
# speedup vs baseline: 1.9085x; 1.7721x over previous
"""Trainium2 Bass kernel for nn_DiarizationLoss (PIT diarization loss).

Strategy (8 NeuronCores, T-sharded data-parallel):
  - Shard T=65536 into 8 slices of TLOC=8192; every core processes all B=32
    samples for its T-slice.  t_loc = 64*p + q  (p partition, q in [0,64)).
  - The masked pairwise BCE cost + VAD BCE reduce to per-sample dot products
    over t.  Per sample the device computes, via TensorEngine PSUM
    accumulation over 64 q-chunks of K=128 partitions:
      stationary (5 cols): [mt_0..3, mask]      mt = labels*mask
      moving     (9 cols): [lp_0..3, lnxv, lq_0..3]
    where lp=ln(p+eps), lq=ln((1+eps)-p), xv=|(1-vad)-pv| (== pv if vad else
    1-pv), lnxv=ln(xv+eps).  16 samples pack per matmul group (stationary
    [128,80] x moving [128,144] -> PSUM [80,144]); 2 groups cover B=32.
  - Preds ship as u16 fixed-point (p*65536): uniform absolute quantization
    keeps both ln(p) and ln(1-p) tails accurate (bf16 near 1.0 does not),
    and ACT's free affine (scale=+/-2^-16) rescales for free.
  - ScalarE computes all Ln columns (the bottleneck engine, ~16.5us/core);
    the VAD column rides inside the lp activation op via a spare 5th pred
    column.  DVE does the label masking; labels+mask ship as one u8 tensor
    cast to bf16 by the SWDGE DMA; pred_vad ships pre-XORed with the vad
    complement mask so xv = (vad ? pv : 1-pv) is a plain u16 copy.
  - Matmuls accumulate into two interleaved PSUM banks (even/odd q) so
    consecutive matmuls never hit the same bank; one copy + one add drain
    them to SBUF.
  - Timing note: tc.For_i puts an all-engine barrier + sem reset between
    loop iterations, so a loop body with a single pass measures the serial
    critical path.  reps>=8 passes per body lets the tile pools pipeline
    DMA/ACT/PE across passes (the real steady-state throughput).
  - Host combines the tiny per-core partial-sum blocks: PIT permutation min
    over the 4x4 cost matrices, means, and the VAD quotient.
"""

import warnings

warnings.filterwarnings("ignore")

from contextlib import ExitStack
from itertools import permutations

import ml_dtypes
import numpy as np

import concourse.bass as bass
import concourse.mybir as mybir
import concourse.tile as tile
from concourse import bacc
from concourse.bass_utils import run_bass_kernel_spmd

F32 = mybir.dt.float32
BF16 = mybir.dt.bfloat16
U8 = mybir.dt.uint8
U16 = mybir.dt.uint16
SC = 1.0 / 65536.0          # u16 fixed-point scale, applied inside ACT
Ln = mybir.ActivationFunctionType.Ln
Alu = mybir.AluOpType

# problem constants (hardcoded per contract)
B, T, S = 32, 65536, 4
EPS = 1e-7
PIT_W, VAD_W = 1.0, 0.5
NCORES = 8
TLOC = T // NCORES          # 8192 timesteps per core
P = 128                     # partitions
Q = TLOC // P               # 64 free chunks per sample
G = 16                      # samples per matmul group
NG = B // G                 # 2 matmul groups
LC = 5                      # stationary cols per sample: mt0..3, mask
MC = 9                      # moving cols per sample: lp0..3, lnxv, lq0..3
PERMS = np.array(list(permutations(range(S))), dtype=np.int64)  # [24, 4]

_CACHE = {}


def _build_nc(reps=1, loop_n=1, skip=()):
    # skip: timing-experiment knob; any of {"act","pe","dve","dmain","dmaout"}
    nc = bacc.Bacc("TRN2", target_bir_lowering=False, debug=False)

    # host pre-laid-out (see _make_in_maps):
    #   ps  u16 [P, B*(5 Q)]  cols 0..3 = preds*65536 (c-major), col 4 scratch
    #   lbm u8  [P, B*(5 Q)]  per-sample [lab0..3, mask]
    #   pv  u16 [P, B*Q]  xv*65536 (pred_vad if vad else 1-pred_vad, via XOR)
    ps_d = nc.dram_tensor("ps", [P, B * 5 * Q], U16, kind="ExternalInput")
    lbm_d = nc.dram_tensor("lbm", [P, B * 5 * Q], U8, kind="ExternalInput")
    pv_d = nc.dram_tensor("pv", [P, B * Q], U16, kind="ExternalInput")
    cst_d = nc.dram_tensor("cst", [P, 2], F32, kind="ExternalInput")
    out_d = nc.dram_tensor("out", [G * LC, NG * G * MC], F32,
                           kind="ExternalOutput")

    with tile.TileContext(nc) as tc, ExitStack() as ctx:
        const_pool = ctx.enter_context(tc.tile_pool(name="const", bufs=1))
        stage_pool = ctx.enter_context(tc.tile_pool(name="stage", bufs=2))
        work_pool = ctx.enter_context(tc.tile_pool(name="work", bufs=1))
        psum_pool = ctx.enter_context(
            tc.tile_pool(name="psum", bufs=2, space="PSUM"))
        out_pool = ctx.enter_context(tc.tile_pool(name="outp", bufs=2))

        lhs_ts = [work_pool.tile([P, G * MC * Q], BF16, tag=f"lhs{g}",
                                 name=f"lhs{g}") for g in range(NG)]
        rhs_ts = [work_pool.tile([P, G * LC * Q], BF16, tag=f"rhs{g}",
                                 name=f"rhs{g}") for g in range(NG)]
        cst_t = const_pool.tile([P, 2], F32, tag="cst")
        nc.sync.dma_start(cst_t[:], cst_d[:])
        eps_ap = cst_t[:, 0:1]       # EPS
        onep_ap = cst_t[:, 1:2]      # 1 + EPS

        def build_pass():
            ps_t = stage_pool.tile([P, B * 5 * Q], U16, tag="ps")
            lbm_t = stage_pool.tile([P, B * 5 * Q], BF16, tag="lbm")
            pv_t = stage_pool.tile([P, B * Q], U16, tag="pv")
            half = B * 5 * Q // 2
            if "dmain" not in skip:
                # ps group-0 half first: unblocks ACT(g0) earliest
                nc.sync.dma_start(ps_t[:, :half], ps_d[:, :half])
                nc.gpsimd.dma_start(lbm_t[:], lbm_d[:])  # u8->bf16 cast DMA
                nc.sync.dma_start(pv_t[:], pv_d[:])
                nc.sync.dma_start(ps_t[:, half:], ps_d[:, half:])

            ps_r = ps_t[:].rearrange("p (s c q) -> p s c q", s=B, c=5, q=Q)
            lbm_r = lbm_t[:].rearrange("p (s c q) -> p s c q", s=B, c=5, q=Q)

            # timing-experiment stubs: satisfy write-before-read tracking
            if "dmain" in skip:
                for t in (ps_t, lbm_t, pv_t):
                    nc.vector.memset(t[:, 0:1], 0)
            if "act" in skip:
                for t in lhs_ts:
                    nc.vector.memset(t[:, 0:1], 0)
            if "dve" in skip:
                for t in rhs_ts:
                    nc.vector.memset(t[:, 0:1], 0)

            ot = out_pool.tile([G * LC, NG * G * MC], F32, tag="ot")
            if "pe" in skip:
                nc.vector.memset(ot[:, 0:1], 0)
            for g in range(NG):
                s0, s1 = g * G, (g + 1) * G
                lhs_r = lhs_ts[g][:].rearrange("p (s c q) -> p s c q",
                                               s=G, c=MC, q=Q)
                rhs_r = rhs_ts[g][:].rearrange("p (s c q) -> p s c q",
                                               s=G, c=LC, q=Q)

                if "dve" not in skip:
                    # xv (= pv if vad else 1-pv, host-XORed u16) into ps col 4
                    nc.vector.tensor_copy(
                        ps_r[:, s0:s1, 4, :],
                        pv_t[:, s0 * Q:s1 * Q].rearrange(
                            "p (s q) -> p s q", s=G, q=Q))
                    # mt = labels * mask ; mask copy
                    nc.vector.tensor_tensor(
                        rhs_r[:, :, 0:4, :], lbm_r[:, s0:s1, 0:4, :],
                        lbm_r[:, s0:s1, 4:5, :].broadcast_to([P, G, 4, Q]),
                        op=Alu.mult)
                    nc.vector.tensor_copy(rhs_r[:, :, 4, :],
                                          lbm_r[:, s0:s1, 4, :])

                if "act" not in skip:
                    # lp0..3 + lnxv, then lq0..3 (scale 2^-16 inside ACT)
                    nc.scalar.activation(lhs_r[:, :, 0:5, :],
                                         ps_r[:, s0:s1, :, :],
                                         Ln, bias=eps_ap, scale=SC)
                    nc.scalar.activation(lhs_r[:, :, 5:9, :],
                                         ps_r[:, s0:s1, 0:4, :],
                                         Ln, bias=onep_ap, scale=-SC)

                # matmul: two interleaved PSUM-bank chains (even/odd q) so
                # consecutive accumulating matmuls never target the same bank
                lhs_f, rhs_f = lhs_ts[g][:], rhs_ts[g][:]
                acc_e = psum_pool.tile([G * LC, G * MC], F32, tag=f"acce{g}",
                                       name=f"acce{g}")
                acc_o = psum_pool.tile([G * LC, G * MC], F32, tag=f"acco{g}",
                                       name=f"acco{g}")
                if "pe" not in skip:
                    for q in range(Q):
                        stat = bass.AP(rhs_f.tensor, rhs_f.offset + q,
                                       [list(rhs_f.ap[0]), [Q, G * LC]])
                        mov = bass.AP(lhs_f.tensor, lhs_f.offset + q,
                                      [list(lhs_f.ap[0]), [Q, G * MC]])
                        acc = acc_e if q % 2 == 0 else acc_o
                        nc.tensor.matmul(acc[:], stat, mov,
                                         start=(q < 2), stop=(q >= Q - 2))
                    osl = ot[:, g * G * MC:(g + 1) * G * MC]
                    nc.vector.tensor_copy(osl, acc_e[:])
                    nc.vector.tensor_tensor(osl, osl, acc_o[:], op=Alu.add)

            if "dmaout" not in skip:
                nc.sync.dma_start(out_d[:], ot[:])

        # reps/loop_n > 1 only for timing-by-differencing in test.py
        if loop_n > 1:
            with tc.For_i(0, loop_n, 1):
                for _ in range(reps):
                    build_pass()
        else:
            for _ in range(reps):
                build_pass()

    nc.compile()
    return nc


def _get_nc(reps=1, loop_n=1, skip=()):
    key = ("nc", reps, loop_n, tuple(skip))
    if key not in _CACHE:
        _CACHE[key] = _build_nc(reps, loop_n, skip)
    return _CACHE[key]


def _quant16(x):
    # fixed-point u16: round(x * 65536) clipped to [0, 65535]
    return np.minimum(np.rint(x * 65536.0), 65535.0).astype(np.uint16)


def _make_in_maps(pred_speakers, pred_vad, labels, vad, lengths):
    lens = np.asarray(lengths, dtype=np.int64)
    ps_all = np.asarray(pred_speakers, np.float32)
    pv_all = np.asarray(pred_vad, np.float32)
    lb_all = np.asarray(labels)
    vd_all = np.asarray(vad)

    in_maps = []
    for c in range(NCORES):
        t0 = c * TLOC
        # [B, TLOC] validity mask for this core's T-slice (exact int math)
        tidx = t0 + np.arange(TLOC, dtype=np.int64)
        mask = (tidx[None, :] < lens[:, None]).astype(np.uint8)  # [B, TLOC]

        # ps: [B, TLOC, S] -> [P, B, 5, Q] u16, col 4 zero (device scratch)
        psc = ps_all[:, t0:t0 + TLOC, :].reshape(B, P, Q, S)
        ps = np.zeros((P, B, 5, Q), dtype=np.uint16)
        ps[:, :, 0:4, :] = _quant16(psc).transpose(1, 0, 3, 2)

        # lbm: u8 [lab0..3, mask] per sample
        lbc = lb_all[:, t0:t0 + TLOC, :].reshape(B, P, Q, S).astype(np.uint8)
        lbm = np.empty((P, B, 5, Q), np.uint8)
        lbm[:, :, 0:4, :] = lbc.transpose(1, 0, 3, 2)
        lbm[:, :, 4, :] = mask.reshape(B, P, Q).transpose(1, 0, 2)

        # xv encoding: pv_u16 XOR ((1-vad)*0xFFFF) == pv if vad else 1-pv
        off = ((1 - vd_all[:, t0:t0 + TLOC].astype(np.int64)) * 65535) \
            .astype(np.uint16)
        pv = np.ascontiguousarray(
            (_quant16(pv_all[:, t0:t0 + TLOC]) ^ off)
            .reshape(B, P, Q).transpose(1, 0, 2)).reshape(P, B * Q)

        cst = np.zeros((P, 2), np.float32)
        cst[:, 0] = EPS
        cst[:, 1] = 1.0 + EPS

        in_maps.append({
            "ps": np.ascontiguousarray(ps.reshape(P, B * 5 * Q)),
            "lbm": np.ascontiguousarray(lbm.reshape(P, B * 5 * Q)),
            "pv": pv,
            "cst": cst,
        })
    return in_maps


def _combine(outs, lengths):
    """Host reduction of per-core partial-sum blocks -> scalar loss."""
    tot = np.zeros((G * LC, NG * G * MC), np.float64)
    for o in outs:
        tot += o.astype(np.float64)

    lens = np.asarray(lengths, dtype=np.float64)
    speaker_sum = 0.0
    vad_num = 0.0
    for b in range(B):
        g, s = b // G, b % G
        blk = tot[LC * s:LC * s + LC,
                  g * G * MC + MC * s:g * G * MC + MC * s + MC]
        P1 = blk[0:4, 0:4].T        # [i, j] = sum lp_i * mt_j
        Q1 = blk[0:4, 5:9].T        # [i, j] = sum lq_i * mt_j
        Q2 = blk[4, 5:9]            # [i]    = sum lq_i * mask
        vad_num += -blk[4, 4]       # sum mask * ln(xv + eps)

        term1 = -(P1 - Q1)          # [4, 4]
        term2 = -Q2                 # [4]
        L = (term1 + term2[:, None]) / lens[b]
        perm_losses = L[np.arange(S)[None, :], PERMS].mean(axis=-1)  # [24]
        speaker_sum += perm_losses.min()

    speaker_loss = speaker_sum / B
    vad_loss = vad_num / lens.sum()
    return np.float32(PIT_W * speaker_loss + VAD_W * vad_loss)


def kernel(pred_speakers, pred_vad, labels, vad, lengths):
    nc = _get_nc()
    in_maps = _make_in_maps(pred_speakers, pred_vad, labels, vad, lengths)
    res = run_bass_kernel_spmd(nc, in_maps, core_ids=list(range(NCORES)))
    outs = [res.results[c]["out"] for c in range(NCORES)]
    return _combine(outs, lengths)


if __name__ == "__main__":
    rng = np.random.default_rng(0)
    inputs = {
        "pred_speakers": rng.random((B, T, S), np.float32),
        "pred_vad": rng.random((B, T), np.float32),
        "labels": rng.integers(0, 2, (B, T, S)).astype(np.float32),
        "vad": rng.integers(0, 2, (B, T)).astype(np.float32),
        "lengths": np.maximum(rng.integers(0, T, B), T // 2).astype(np.int64),
    }
    print("loss:", kernel(**inputs))


# revision 28
# speedup vs baseline: 1.9287x; 1.0106x over previous
"""Trainium2 Bass kernel for nn_DiarizationLoss (PIT diarization loss).

Strategy (8 NeuronCores, T-sharded data-parallel):
  - Shard T=65536 into 8 slices of TLOC=8192; every core processes all B=32
    samples for its T-slice.  t_loc = 64*p + q  (p partition, q in [0,64)).
  - The masked pairwise BCE cost + VAD BCE reduce to per-sample dot products
    over t.  Per sample the device computes, via TensorEngine PSUM
    accumulation over 64 q-chunks of K=128 partitions:
      stationary (5 cols): [mt_0..3, mask]      mt = labels*mask
      moving     (9 cols): [lp_0..3, lnxv, lq_0..3]
    where lp=ln(p+eps), lq=ln((1+eps)-p), xv=|(1-vad)-pv| (== pv if vad else
    1-pv), lnxv=ln(xv+eps).  16 samples pack per matmul group (stationary
    [128,80] x moving [128,144] -> PSUM [80,144]); 2 groups cover B=32.
  - Preds ship as u16 fixed-point (p*65536): uniform absolute quantization
    keeps both ln(p) and ln(1-p) tails accurate (bf16 near 1.0 does not),
    and ACT's free affine (scale=+/-2^-16) rescales for free.
  - ScalarE computes all Ln columns (the bottleneck engine, ~16.5us/core);
    the VAD column rides inside the lp activation op via a spare 5th pred
    column.  DVE does the label masking; labels+mask ship as one u8 tensor
    cast to bf16 by the SWDGE DMA; pred_vad ships pre-XORed with the vad
    complement mask so xv = (vad ? pv : 1-pv) is a plain u16 copy.
  - Matmuls accumulate into two interleaved PSUM banks (even/odd q) so
    consecutive matmuls never hit the same bank; one copy + one add drain
    them to SBUF.
  - Timing note: tc.For_i puts an all-engine barrier + sem reset between
    loop iterations, so a loop body with a single pass measures the serial
    critical path.  reps>=8 passes per body lets the tile pools pipeline
    DMA/ACT/PE across passes (the real steady-state throughput).
  - Host combines the tiny per-core partial-sum blocks: PIT permutation min
    over the 4x4 cost matrices, means, and the VAD quotient.
"""

import warnings

warnings.filterwarnings("ignore")

from contextlib import ExitStack
from itertools import permutations

import numpy as np

import concourse.bass as bass
import concourse.mybir as mybir
import concourse.tile as tile
from concourse import bacc
from concourse.bass_utils import run_bass_kernel_spmd

F32 = mybir.dt.float32
BF16 = mybir.dt.bfloat16
U8 = mybir.dt.uint8
U16 = mybir.dt.uint16
SC = 1.0 / 65536.0          # u16 fixed-point scale, applied inside ACT
Ln = mybir.ActivationFunctionType.Ln
Alu = mybir.AluOpType

# problem constants (hardcoded per contract)
B, T, S = 32, 65536, 4
EPS = 1e-7
PIT_W, VAD_W = 1.0, 0.5
NCORES = 8
TLOC = T // NCORES          # 8192 timesteps per core
P = 128                     # partitions
Q = TLOC // P               # 64 free chunks per sample
G = 16                      # samples per matmul group
NG = B // G                 # 2 matmul groups
LC = 5                      # live stationary cols per sample: mt0..3, mask
LCP = 8                     # padded stationary cols/sample -> 128/chunk (FWL)
MC = 9                      # moving cols per sample: lp0..3, lnxv, lq0..3
PERMS = np.array(list(permutations(range(S))), dtype=np.int64)  # [24, 4]

_CACHE = {}


def _build_nc(reps=1, loop_n=1, skip=()):
    # skip: timing-experiment knob; any of {"act","pe","dve","dmain","dmaout"}
    nc = bacc.Bacc("TRN2", target_bir_lowering=False, debug=False)

    # host pre-laid-out (see _make_in_maps):
    #   ps  u16 [P, B*(5 Q)]  cols 0..3 = preds*65536 (c-major), col 4 scratch
    #   lbm u8  [P, B*(5 Q)]  per-sample [lab0..3, mask]
    #   pv  u16 [P, B*Q]  xv*65536 (pred_vad if vad else 1-pred_vad, via XOR)
    ps_d = nc.dram_tensor("ps", [P, B * 5 * Q], U16, kind="ExternalInput")
    lbm_d = nc.dram_tensor("lbm", [P, B * 5 * Q], U8, kind="ExternalInput")
    pv_d = nc.dram_tensor("pv", [P, B * Q], U16, kind="ExternalInput")
    cst_d = nc.dram_tensor("cst", [P, 2], F32, kind="ExternalInput")
    out_d = nc.dram_tensor("out", [G * LCP, NG * G * MC], F32,
                           kind="ExternalOutput")

    with tile.TileContext(nc) as tc, ExitStack() as ctx:
        const_pool = ctx.enter_context(tc.tile_pool(name="const", bufs=1))
        stage_pool = ctx.enter_context(tc.tile_pool(name="stage", bufs=2))
        work_pool = ctx.enter_context(tc.tile_pool(name="work", bufs=1))
        psum_pool = ctx.enter_context(
            tc.tile_pool(name="psum", bufs=2, space="PSUM"))
        out_pool = ctx.enter_context(tc.tile_pool(name="outp", bufs=2))

        # chunk-contiguous layouts: [P, (q, s, c)] so each matmul chunk's
        # stationary (128 cols incl. zero pads -> FWL) and moving (144 cols)
        # are contiguous in SBUF
        lhs_ts = [work_pool.tile([P, Q * G * MC], BF16, tag=f"lhs{g}",
                                 name=f"lhs{g}") for g in range(NG)]
        rhs_ts = [work_pool.tile([P, Q * G * LCP], BF16, tag=f"rhs{g}",
                                 name=f"rhs{g}") for g in range(NG)]
        for t in rhs_ts:            # zero once: pad cols stay 0 forever
            nc.vector.memset(t[:], 0)
        cst_t = const_pool.tile([P, 2], F32, tag="cst")
        nc.sync.dma_start(cst_t[:], cst_d[:])
        eps_ap = cst_t[:, 0:1]       # EPS
        onep_ap = cst_t[:, 1:2]      # 1 + EPS

        def build_pass():
            ps_t = stage_pool.tile([P, B * 5 * Q], U16, tag="ps")
            lbm_t = stage_pool.tile([P, B * 5 * Q], BF16, tag="lbm")
            pv_t = stage_pool.tile([P, B * Q], U16, tag="pv")
            half = B * 5 * Q // 2
            if "dmain" not in skip:
                # ps group-0 half first: unblocks ACT(g0) earliest
                nc.sync.dma_start(ps_t[:, :half], ps_d[:, :half])
                nc.gpsimd.dma_start(lbm_t[:], lbm_d[:])  # u8->bf16 cast DMA
                nc.sync.dma_start(pv_t[:], pv_d[:])
                nc.sync.dma_start(ps_t[:, half:], ps_d[:, half:])

            ps_r = ps_t[:].rearrange("p (s c q) -> p s c q", s=B, c=5, q=Q)
            ps_q = ps_t[:].rearrange("p (s c q) -> p q s c", s=B, c=5, q=Q)
            lbm_r = lbm_t[:].rearrange("p (q s c) -> p q s c", q=Q, s=B, c=5)

            # timing-experiment stubs: satisfy write-before-read tracking
            if "dmain" in skip:
                for t in (ps_t, lbm_t, pv_t):
                    nc.vector.memset(t[:, 0:1], 0)
            if "act" in skip:
                for t in lhs_ts:
                    nc.vector.memset(t[:, 0:1], 0)
            if "dve" in skip:
                for t in rhs_ts:
                    nc.vector.memset(t[:, 0:1], 0)

            ot = out_pool.tile([G * LCP, NG * G * MC], F32, tag="ot")
            if "pe" in skip:
                nc.vector.memset(ot[:, 0:1], 0)
            for g in range(NG):
                s0, s1 = g * G, (g + 1) * G
                lhs_r = lhs_ts[g][:].rearrange("p (q s c) -> p q s c",
                                               q=Q, s=G, c=MC)
                rhs_r = rhs_ts[g][:].rearrange("p (q s c) -> p q s c",
                                               q=Q, s=G, c=LCP)

                if "dve" not in skip:
                    # xv (= pv if vad else 1-pv, host-XORed u16) into ps col 4
                    nc.vector.tensor_copy(
                        ps_r[:, s0:s1, 4, :],
                        pv_t[:, s0 * Q:s1 * Q].rearrange(
                            "p (s q) -> p s q", s=G, q=Q))
                    # mt = labels * mask ; mask copy
                    nc.vector.tensor_tensor(
                        rhs_r[:, :, :, 0:4], lbm_r[:, :, s0:s1, 0:4],
                        lbm_r[:, :, s0:s1, 4:5].broadcast_to([P, Q, G, 4]),
                        op=Alu.mult)
                    nc.vector.tensor_copy(rhs_r[:, :, :, 4],
                                          lbm_r[:, :, s0:s1, 4])

                if "act" not in skip:
                    # lp0..3 + lnxv, then lq0..3 (scale 2^-16 inside ACT)
                    nc.scalar.activation(lhs_r[:, :, :, 0:5],
                                         ps_q[:, :, s0:s1, :],
                                         Ln, bias=eps_ap, scale=SC)
                    nc.scalar.activation(lhs_r[:, :, :, 5:9],
                                         ps_q[:, :, s0:s1, 0:4],
                                         Ln, bias=onep_ap, scale=-SC)

                # matmul: two interleaved PSUM-bank chains (even/odd q) so
                # consecutive accumulating matmuls never target the same bank
                lhs_f, rhs_f = lhs_ts[g][:], rhs_ts[g][:]
                acc_e = psum_pool.tile([G * LCP, G * MC], F32, tag=f"acce{g}",
                                       name=f"acce{g}")
                acc_o = psum_pool.tile([G * LCP, G * MC], F32, tag=f"acco{g}",
                                       name=f"acco{g}")
                if "pe" not in skip:
                    for q in range(Q):
                        stat = bass.AP(rhs_f.tensor,
                                       rhs_f.offset + q * (G * LCP),
                                       [list(rhs_f.ap[0]), [1, G * LCP]])
                        mov = bass.AP(lhs_f.tensor,
                                      lhs_f.offset + q * (G * MC),
                                      [list(lhs_f.ap[0]), [1, G * MC]])
                        acc = acc_e if q % 2 == 0 else acc_o
                        nc.tensor.matmul(acc[:], stat, mov,
                                         start=(q < 2), stop=(q >= Q - 2))
                    osl = ot[:, g * G * MC:(g + 1) * G * MC]
                    nc.vector.tensor_copy(osl, acc_e[:])
                    nc.vector.tensor_tensor(osl, osl, acc_o[:], op=Alu.add)

            if "dmaout" not in skip:
                nc.sync.dma_start(out_d[:], ot[:])

        # reps/loop_n > 1 only for timing-by-differencing in test.py
        if loop_n > 1:
            with tc.For_i(0, loop_n, 1):
                for _ in range(reps):
                    build_pass()
        else:
            for _ in range(reps):
                build_pass()

    nc.compile()
    return nc


def _get_nc(reps=1, loop_n=1, skip=()):
    key = ("nc", reps, loop_n, tuple(skip))
    if key not in _CACHE:
        _CACHE[key] = _build_nc(reps, loop_n, skip)
    return _CACHE[key]


def _quant16(x):
    # fixed-point u16: round(x * 65536) clipped to [0, 65535]
    return np.minimum(np.rint(x * 65536.0), 65535.0).astype(np.uint16)


def _make_in_maps(pred_speakers, pred_vad, labels, vad, lengths):
    lens = np.asarray(lengths, dtype=np.int64)
    ps_all = np.asarray(pred_speakers, np.float32)
    pv_all = np.asarray(pred_vad, np.float32)
    lb_all = np.asarray(labels)
    vd_all = np.asarray(vad)

    in_maps = []
    for c in range(NCORES):
        t0 = c * TLOC
        # [B, TLOC] validity mask for this core's T-slice (exact int math)
        tidx = t0 + np.arange(TLOC, dtype=np.int64)
        mask = (tidx[None, :] < lens[:, None]).astype(np.uint8)  # [B, TLOC]

        # ps: [B, TLOC, S] -> [P, B, 5, Q] u16, col 4 zero (device scratch)
        psc = ps_all[:, t0:t0 + TLOC, :].reshape(B, P, Q, S)
        ps = np.zeros((P, B, 5, Q), dtype=np.uint16)
        ps[:, :, 0:4, :] = _quant16(psc).transpose(1, 0, 3, 2)

        # lbm: u8 [P, Q, B, 5] = per chunk q, per sample: [lab0..3, mask]
        lbc = lb_all[:, t0:t0 + TLOC, :].reshape(B, P, Q, S).astype(np.uint8)
        lbm = np.empty((P, Q, B, 5), np.uint8)
        lbm[:, :, :, 0:4] = lbc.transpose(1, 2, 0, 3)
        lbm[:, :, :, 4] = mask.reshape(B, P, Q).transpose(1, 2, 0)

        # xv encoding: pv_u16 XOR ((1-vad)*0xFFFF) == pv if vad else 1-pv
        off = ((1 - vd_all[:, t0:t0 + TLOC].astype(np.int64)) * 65535) \
            .astype(np.uint16)
        pv = np.ascontiguousarray(
            (_quant16(pv_all[:, t0:t0 + TLOC]) ^ off)
            .reshape(B, P, Q).transpose(1, 0, 2)).reshape(P, B * Q)

        cst = np.zeros((P, 2), np.float32)
        cst[:, 0] = EPS
        cst[:, 1] = 1.0 + EPS

        in_maps.append({
            "ps": np.ascontiguousarray(ps.reshape(P, B * 5 * Q)),
            "lbm": np.ascontiguousarray(lbm.reshape(P, B * 5 * Q)),
            "pv": pv,
            "cst": cst,
        })
    return in_maps


def _combine(outs, lengths):
    """Host reduction of per-core partial-sum blocks -> scalar loss."""
    tot = np.zeros((G * LCP, NG * G * MC), np.float64)
    for o in outs:
        tot += o.astype(np.float64)

    lens = np.asarray(lengths, dtype=np.float64)
    speaker_sum = 0.0
    vad_num = 0.0
    for b in range(B):
        g, s = b // G, b % G
        blk = tot[LCP * s:LCP * s + LC,
                  g * G * MC + MC * s:g * G * MC + MC * s + MC]
        P1 = blk[0:4, 0:4].T        # [i, j] = sum lp_i * mt_j
        Q1 = blk[0:4, 5:9].T        # [i, j] = sum lq_i * mt_j
        Q2 = blk[4, 5:9]            # [i]    = sum lq_i * mask
        vad_num += -blk[4, 4]       # sum mask * ln(xv + eps)

        term1 = -(P1 - Q1)          # [4, 4]
        term2 = -Q2                 # [4]
        L = (term1 + term2[:, None]) / lens[b]
        perm_losses = L[np.arange(S)[None, :], PERMS].mean(axis=-1)  # [24]
        speaker_sum += perm_losses.min()

    speaker_loss = speaker_sum / B
    vad_loss = vad_num / lens.sum()
    return np.float32(PIT_W * speaker_loss + VAD_W * vad_loss)


def kernel(pred_speakers, pred_vad, labels, vad, lengths):
    nc = _get_nc()
    in_maps = _make_in_maps(pred_speakers, pred_vad, labels, vad, lengths)
    res = run_bass_kernel_spmd(nc, in_maps, core_ids=list(range(NCORES)))
    outs = [res.results[c]["out"] for c in range(NCORES)]
    return _combine(outs, lengths)


if __name__ == "__main__":
    rng = np.random.default_rng(0)
    inputs = {
        "pred_speakers": rng.random((B, T, S), np.float32),
        "pred_vad": rng.random((B, T), np.float32),
        "labels": rng.integers(0, 2, (B, T, S)).astype(np.float32),
        "vad": rng.integers(0, 2, (B, T)).astype(np.float32),
        "lengths": np.maximum(rng.integers(0, T, B), T // 2).astype(np.int64),
    }
    print("loss:", kernel(**inputs))
